# revision 1
# baseline (speedup 1.0000x reference)
"""Category-specific linear on 8 trn2 cores — hidden-dim sharding, resident W.

Changes vs v1:
- One 4-bank PSUM tile per sample ([128, 4(m), 512]); all 32 MMs of a sample
  accumulate into it, then ONE DVE copy + ONE 1 MiB output DMA per sample.
- x loaded in pairs of samples (1 MiB DMAs).
- Output DRAM layout [B, 128, MO, NSH]; host unpacks.
"""

import numpy as np
import ml_dtypes

B = 64
S = 512
DIN = 1024
DH = 4096
C = 16
NCORES = 8
NSH = DH // NCORES   # 512
P = 128
KO = DIN // P        # 8
MO = S // P          # 4

LAST_RESULTS = None


def _build_program(cats):
    import concourse.bacc as bacc
    import concourse.mybir as mybir
    import concourse.tile as tile

    nc = bacc.Bacc("TRN2", target_bir_lowering=False)

    assert B % 2 == 0
    x_d = nc.dram_tensor("x", (B // 2, P, 2, KO, S), mybir.dt.bfloat16,
                         kind="ExternalInput")
    w_d = nc.dram_tensor("w", (C, P, KO, NSH), mybir.dt.bfloat16,
                         kind="ExternalInput")
    out_d = nc.dram_tensor("out", (B, P, MO, NSH), mybir.dt.float32,
                           kind="ExternalOutput")

    # Category-grouped processing order: samples of the same category are
    # consecutive, so each category's W slice is prefetched once and covered
    # by ~n_c * 6.9us of compute. proc_order[i] = original sample index.
    used = []
    for c in cats:
        if c not in used:
            used.append(c)
    proc_order = [j for c in used for j in range(B) if cats[j] == c]
    # group g -> category used[g]; first processed position of each group
    group_start = {}
    for i, j in enumerate(proc_order):
        group_start.setdefault(cats[j], i)
    n_groups = len(used)

    with tile.TileContext(nc) as tc:
        with (
            tc.tile_pool(name="wpool", bufs=1) as wp,
            tc.tile_pool(name="xpool", bufs=3) as xb,
            tc.tile_pool(name="opool", bufs=3) as ob,
            tc.tile_pool(name="psum", bufs=2, space="PSUM") as ps,
        ):
            w_tiles = {}

            def emit_w(g):
                if g >= n_groups:
                    return
                c = used[g]
                t = wp.tile([P, KO, NSH], mybir.dt.bfloat16, tag=f"w{c}")
                nc.sync.dma_start(t[:], w_d[c])
                w_tiles[c] = t

            # PE warmup: 16 dummy matmuls on zeroed scratch tiles with no DMA
            # deps. They run during the unavoidable initial DMA wait, so the
            # HAM clock-gate reaches 8/8 before real data lands.
            warm_l = wp.tile([P, P], mybir.dt.bfloat16, tag="warm_l")
            warm_r = wp.tile([P, NSH], mybir.dt.bfloat16, tag="warm_r")
            nc.any.memzero(warm_l[:])
            nc.any.memzero(warm_r[:])
            warm_p = ps.tile([P, MO, NSH], mybir.dt.float32, tag="ps")
            for _ in range(16):
                nc.tensor.matmul(warm_p[:, 0, :], warm_l[:], warm_r[:],
                                 start=True, stop=True)

            # First x pair is issued BEFORE any weight DMA: each dma_start
            # costs ~0.65us of issue time on the sync sequencer and issue
            # order decides who gets HBM first — the 1 MiB x pair is the
            # critical pole for the first matmul.
            xt_first = xb.tile([P, 2, KO, S], mybir.dt.bfloat16, tag="x")
            nc.sync.dma_start(xt_first[:], x_d[0])

            # Group 0's W arrives as 8 per-ko pieces (128 KiB each) so the
            # first real matmul's weight dep is one piece, not the full 1 MiB.
            c0 = used[0]
            w0k = []
            for k in range(KO):
                twk = wp.tile([P, NSH], mybir.dt.bfloat16, tag=f"w{c0}_k{k}")
                nc.sync.dma_start(twk[:], w_d[c0, :, k, :])
                w0k.append(twk)

            # Seed the next two groups' weights; prefetch the rest two
            # groups ahead of use.
            for g in range(1, min(3, n_groups)):
                emit_w(g)

            for i in range(B):
                j = proc_order[i]
                c = cats[j]
                g = used.index(c)
                if group_start[c] == i and g + 2 < n_groups and used[g + 2] not in w_tiles:
                    emit_w(g + 2)
                if i % 2 == 0:
                    if i == 0:
                        xt = xt_first
                    else:
                        xt = xb.tile([P, 2, KO, S], mybir.dt.bfloat16, tag="x")
                        nc.sync.dma_start(xt[:], x_d[i // 2])
                pt = ps.tile([P, MO, NSH], mybir.dt.float32, tag="ps")
                for m in range(MO):
                    for k in range(KO):
                        rhs = w0k[k][:] if c == c0 else w_tiles[c][:, k, :]
                        nc.tensor.matmul(
                            pt[:, m, :],
                            xt[:, i % 2, k, m * P:(m + 1) * P],
                            rhs,
                            start=(k == 0),
                            stop=(k == KO - 1),
                        )
                ot = ob.tile([P, MO, NSH], mybir.dt.float32, tag="o")
                nc.vector.tensor_copy(ot[:], pt[:])
                nc.sync.dma_start(out_d[i], ot[:])

    nc.compile()
    return nc


def kernel(x, cat_ids, W, b):
    global LAST_RESULTS
    from concourse import bass_utils

    x = np.asarray(x, dtype=np.float32)
    cat_ids_np = np.asarray(cat_ids).astype(np.int64)
    W = np.asarray(W, dtype=np.float32)
    b = np.asarray(b, dtype=np.float32)
    cats = [int(c) for c in cat_ids_np]

    # Same category-grouped order the program bakes in.
    used = []
    for c in cats:
        if c not in used:
            used.append(c)
    proc_order = [j for c in used for j in range(B) if cats[j] == c]

    # x: [B,S,DIN] -> [B,P(q),KO,S] bf16, in processed order, paired.
    xp = np.ascontiguousarray(
        x.reshape(B, S, KO, P).transpose(0, 3, 2, 1)
    ).astype(ml_dtypes.bfloat16)[proc_order]
    xp = np.ascontiguousarray(
        xp.reshape(B // 2, 2, P, KO, S).transpose(0, 2, 1, 3, 4)
    )

    in_maps = []
    for core in range(NCORES):
        Wc = W[:, :, core * NSH:(core + 1) * NSH]
        Wp = np.ascontiguousarray(
            Wc.reshape(C, KO, P, NSH).transpose(0, 2, 1, 3)
        ).astype(ml_dtypes.bfloat16)
        in_maps.append({"x": xp, "w": Wp})

    nc = _build_program(cats)
    res = bass_utils.run_bass_kernel_spmd(
        nc, in_maps, core_ids=list(range(NCORES))
    )
    LAST_RESULTS = res

    inv = np.argsort(np.asarray(proc_order))
    out = np.empty((B, S, DH), dtype=np.float32)
    for core in range(NCORES):
        oc = res.results[core]["out"]                    # [B(proc), P, MO, NSH]
        # out[proc_order[i], m*128+p, n] = oc[i, p, m, n]
        oc = oc.reshape(B, P, MO, NSH).transpose(0, 2, 1, 3).reshape(B, S, NSH)
        out[:, :, core * NSH:(core + 1) * NSH] = oc[inv]

    if b.any():
        out += b[cats][:, None, :]
    return out

